# revision 47
# baseline (speedup 1.0000x reference)
"""LoFTR cross-attention on 8 Trainium2 NeuronCores.

Problem: x [2, 4096, 256], source [2, 6144, 256], Wq/Wk/Wv [256, 256] (torch
Linear convention, y = x @ W.T), 8 heads x 32 dims, softmax cross-attention,
output [2, 4096, 256] fp32.

Sharding: 16 (batch, head) pairs over 8 cores -> each core owns one batch b
and two adjacent heads {2p, 2p+1}. Per core:
  q = x[b] @ Wq_h.T        [4096, 32] per head
  k = source[b] @ Wk_h.T   [6144, 32]
  v = source[b] @ Wv_h.T   [6144, 32]
  out = softmax(q k^T / sqrt(32)) v

Engine-balance strategy (PE is the floor at ~340us bf16; everything else
must hide under it):
  - Affine fold: host scales Wq/Wk by alpha = sqrt(p/sqrt(32)) and the kT/qT
    tiles carry an extra contraction row (bias q0 x ones), so the scores
    matmul lands u = p*x + q0 in PSUM (x = scores/sqrt(32)). Contraction rows
    are free on PE (cost = moving-free size only).
  - exp is split by PSUM group: 14/16 groups go to ScalarE as
    exp(u*(1/p) + bias) straight from PSUM (~1 col/ns); 2/16 groups go to DVE
    as the 3-op poly lam*e^x ~ ((u^2)+a)^2: square (1x from PSUM), +a (bf16
    TSP at 4x), square (bf16 TT at 2x). This drops DVE from 380us to ~160us.
  - Projections compute BOTH heads in one matmul (W pack [128, 96], head 1 at
    partition 64 so contraction-33 scores matmuls can use tile_position
    (64,0)). No 4x row-replication (row tiling does not pay on this HW).
  - attn @ v uses v as the stationary operand with a ones column appended
    (lhsT = [v | 1] [128 kpos, 33]) accumulating over 48 k-chunks, so the
    softmax denominator falls out as PSUM row 32 for free.
  - A small PE transpose + DVE reciprocal/mul normalizes and lands the output
    in natural [qpos, dhead] layout for one big DMA out.
  - Bias-row/ones memsets go to the idle Pool (gpsimd) engine.
"""

import numpy as np

B = 2
L1 = 4096
L2 = 6144
D = 256
NHEAD = 8
DH = 32
HEADS_PER_CORE = 2
N_CORES = 8
QB = 512                 # query block (free dim of scores matmuls)
NQ = L1 // QB            # 8 query chunks
NK = L2 // 128           # 48 key chunks of 128
GC = 2                   # key chunks per PSUM scores group
NG = NK // GC            # 24 groups
POLY_GROUPS = (1, 5, 9, 13, 17, 21)  # groups whose exp runs as the DVE poly
                          # (spread out so the DVE work never bursts)
SCALE = 1.0 / np.sqrt(DH)
VW = DH + 1              # v columns + ones column
VP = 48                  # VW padded to the DMA-transpose xbar granularity

# exp approximation (see fit: lam*e^x ~ ((p*x+q0)^2+a)^2, u = p*x+q0 from the
# matmul; ScalarE computes lam*e^x = exp(u/p + (lnl - q0/p)) exactly)
PP = 0.3442070165780399
Q0 = 0.75                # bf16-exact bias row value
A3 = 0.44335199901470956
LNL = 0.005094758864906013
ACT_SCALE = 1.0 / PP
ACT_BIAS = LNL - Q0 / PP

_CACHE = {}


_MAXW = 1  # this walrus build accepts only one sync wait per instruction


def _patch_tile_drain():
    """This walrus build rejects instructions carrying more than one sync
    wait. Tile's sem-assignment freely puts several waits on one instruction
    (and the kernel-tail drain waits on every logical processor). Split the
    excess onto injected same-engine nops placed immediately before the
    overloaded instruction — engines are in-order, so semantics are kept."""
    import concourse.mybir as mybir
    import concourse.tile as tile
    from concourse.vector_clock import ScopedClock

    if getattr(tile.TileContext, "_drain_split_patched", False):
        return

    orig_lower = tile.TileContext._lower_ordered_insts
    counter = [0]

    def _split_waits(self, ordered):
        for bb_name, insts in ordered.items():
            out = []
            for inst in insts:
                si = inst.sync_info
                waits = list(si.on_wait) if si and si.on_wait else []
                if len(waits) > _MAXW:
                    for j in range(0, len(waits) - _MAXW, _MAXW):
                        counter[0] += 1
                        nop = mybir.InstNoOp(name=f"waitsplit-{counter[0]}")
                        nop.engine = inst.engine
                        nop.sync_info = mybir.SyncInfo(
                            on_wait=waits[j:j + _MAXW], on_update=[]
                        )
                        if inst.debug is not None:
                            nop.debug = inst.debug
                        out.append(nop)
                    inst.sync_info = mybir.SyncInfo(
                        on_wait=waits[len(waits) - _MAXW:],
                        on_update=list(si.on_update) if si.on_update else [],
                    )
                out.append(inst)
            ordered[bb_name] = out
        return orig_lower(self, ordered)

    tile.TileContext._lower_ordered_insts = _split_waits

    def _drain_and_barrier(self, tick_clock, wait_clock):
        carrier = self.nc.sync.nop(nofuse=True)
        wait_clock.add_sem_waits(
            carrier.ins, ScopedClock({None: tick_clock.global_clock})
        )
        si = carrier.ins.sync_info
        waits = list(si.on_wait) if si and si.on_wait else []
        carrier.ins.sync_info = mybir.SyncInfo(on_wait=waits[:_MAXW], on_update=[])
        for j in range(_MAXW, len(waits), _MAXW):
            nop = self.nc.sync.nop(nofuse=True)
            nop.ins.sync_info = mybir.SyncInfo(on_wait=waits[j:j + _MAXW], on_update=[])
        self.nc.sync.drain()
        self.nc.all_engine_barrier()
        assert self.sems is not None
        popped = self.nc._tile_sem_poison_stack.pop()
        assert popped is self._sem_poison
        self.nc.clear_and_free_semaphores(list(self.sems.allocated().values()))
        self.nc.all_engine_barrier()

    tile.TileContext._drain_and_barrier = _drain_and_barrier
    tile.TileContext._drain_split_patched = True


def _build():
    import concourse.bass as bass
    import concourse.mybir as mybir
    import concourse.tile as tile
    from concourse.masks import make_identity

    _patch_tile_drain()

    fp32 = mybir.dt.float32
    bf16 = mybir.dt.bfloat16
    Alu = mybir.AluOpType

    nc = bass.Bass("TRN2")
    xT_d = nc.dram_tensor("xT", [D, L1], bf16, kind="ExternalInput")
    sT_d = nc.dram_tensor("srcT", [D, L2], bf16, kind="ExternalInput")
    wq_d = nc.dram_tensor("wq", [128, 192], bf16, kind="ExternalInput")
    wk_d = nc.dram_tensor("wk", [128, 192], bf16, kind="ExternalInput")
    wv_d = nc.dram_tensor("wv", [128, 128], bf16, kind="ExternalInput")
    out_d = nc.dram_tensor("out", [L1, 2 * DH], fp32, kind="ExternalOutput")

    with tile.TileContext(nc) as tc:
        with (
            tc.tile_pool(name="fixed", bufs=1) as fixed,
            tc.tile_pool(name="epool", bufs=24) as epool,
            tc.tile_pool(name="poly", bufs=3) as poly,
            tc.tile_pool(name="tmp", bufs=3) as tmp,
            tc.tile_pool(name="ps_sc", bufs=3, space="PSUM") as ps_sc,
            tc.tile_pool(name="ps_sm", bufs=2, space="PSUM") as ps_sm,
        ):
            # ---- input DMAs (weights first; big tensors split across queues)
            wq = fixed.tile([128, 192], bf16, tag="wq", name="wq_sb")
            wk = fixed.tile([128, 192], bf16, tag="wk", name="wk_sb")
            wv = fixed.tile([128, 128], bf16, tag="wv", name="wv_sb")
            nc.scalar.dma_start(out=wk, in_=wk_d[:, :])
            nc.scalar.dma_start(out=wq, in_=wq_d[:, :])
            nc.scalar.dma_start(out=wv, in_=wv_d[:, :])
            xT = [fixed.tile([128, L1], bf16, tag=f"xT{t}", name=f"xT{t}") for t in range(2)]
            sT = [fixed.tile([128, L2], bf16, tag=f"sT{t}", name=f"sT{t}") for t in range(2)]
            # sT (feeds k_proj/scores first) on the SP queue with the first
            # quarter split into eighths so k_proj(0) starts ~1.3us in; xT on
            # the Activation queue right behind the weights.
            E8 = L2 // 8
            for j in range(2):
                for t in range(2):
                    nc.sync.dma_start(
                        out=sT[t][:, j * E8:(j + 1) * E8],
                        in_=sT_d[t * 128:(t + 1) * 128, j * E8:(j + 1) * E8],
                    )
            for j in (1, 2, 3):
                for t in range(2):
                    a, b = j * (L2 // 4), (j + 1) * (L2 // 4)
                    nc.sync.dma_start(out=sT[t][:, a:b], in_=sT_d[t * 128:(t + 1) * 128, a:b])
            for j in range(4):
                for t in range(2):
                    a, b = j * (L1 // 4), (j + 1) * (L1 // 4)
                    nc.scalar.dma_start(out=xT[t][:, a:b], in_=xT_d[t * 128:(t + 1) * 128, a:b])

            ident = fixed.tile([VW, VW], fp32, tag="ident", name="ident")
            make_identity(nc, ident)

            # h0 rows 0:32 (+bias row 32), h1 rows 64:96 (+bias row 96)
            qT = fixed.tile([97, L1], bf16, tag="qT", name="qT")
            kT = fixed.tile([97, L2], bf16, tag="kT", name="kT")
            vext = fixed.tile([128, NK, 2 * VP], bf16, tag="v", name="v")
            out_sb = fixed.tile([128, L1 // 128, 2 * DH], fp32, tag="osb", name="osb")

            # constant rows/cols: split Pool/DVE so head-0's rows are ready
            # before the first scores matmul
            bias_ap = fixed.tile([128, 1], fp32, tag="bias", name="act_bias")
            nc.gpsimd.memset(bias_ap[:, :], float(ACT_BIAS))
            nc.gpsimd.memset(vext[:, :, DH:VP], 0.0)
            nc.gpsimd.memset(vext[:, :, VP + DH:2 * VP], 0.0)
            nc.gpsimd.memset(vext[:, :, DH:DH + 1], 1.0)
            nc.gpsimd.memset(vext[:, :, VP + DH:VP + DH + 1], 1.0)

            def k_proj(cchunk):
                psk = ps_sm.tile([128, QB], fp32, tag="small", name="ps_small")
                for t in range(2):
                    nc.tensor.matmul(
                        psk[0:96, :],
                        wk[:, t * 96:(t + 1) * 96],
                        sT[t][:, cchunk * QB:(cchunk + 1) * QB],
                        start=(t == 0), stop=(t == 1),
                    )
                # one copy covers both heads (rows 32:64 are zeros from the
                # zero pack columns); the h0 bias row is then re-stamped
                sl = slice(cchunk * QB, (cchunk + 1) * QB)
                nc.vector.tensor_copy(kT[0:96, sl], psk[0:96, :])
                nc.gpsimd.memset(kT[32:33, sl], float(Q0))
                nc.gpsimd.memset(kT[96:97, sl], float(Q0))

            def q_proj(cchunk):
                psq = ps_sm.tile([128, QB], fp32, tag="small", name="ps_small")
                for t in range(2):
                    nc.tensor.matmul(
                        psq[0:96, :],
                        wq[:, t * 96:(t + 1) * 96],
                        xT[t][:, cchunk * QB:(cchunk + 1) * QB],
                        start=(t == 0), stop=(t == 1),
                    )
                sl = slice(cchunk * QB, (cchunk + 1) * QB)
                nc.vector.tensor_copy(qT[0:96, sl], psq[0:96, :])
                nc.gpsimd.memset(qT[32:33, sl], 1.0)
                nc.gpsimd.memset(qT[96:97, sl], 1.0)

            def v_proj(m):
                psv = ps_sm.tile([128, 64], fp32, tag="small", name="ps_small")
                for t in range(2):
                    nc.tensor.matmul(
                        psv[:, :],
                        sT[t][:, m * 128:(m + 1) * 128],
                        wv[:, t * 64:(t + 1) * 64],
                        start=(t == 0), stop=(t == 1),
                    )
                # both heads in one strided copy: cols {0:32, 48:80}
                nc.vector.tensor_copy(
                    vext[:, m, :].rearrange("p (two vp) -> p two vp", two=2)[:, :, 0:DH],
                    psv[:, :].rearrange("p (two dh) -> p two dh", two=2),
                )

            def scores_group(h, c, g, poly_set=POLY_GROUPS):
                base = 64 * h
                ps = ps_sc.tile([128, GC * QB], fp32, tag="sc", name="ps_sc_t")
                for i in range(GC):
                    m = g * GC + i
                    nc.tensor.matmul(
                        ps[:, i * QB:(i + 1) * QB],
                        kT[base:base + 33, m * 128:(m + 1) * 128],
                        qT[base:base + 33, c * QB:(c + 1) * QB],
                        start=True, stop=True,
                        tile_position=(base, 0),
                    )
                et = epool.tile([128, GC * QB], bf16, tag="E", name="e_t")
                if g not in poly_set:
                    nc.scalar.activation(
                        et[:, :], ps[:, :],
                        mybir.ActivationFunctionType.Exp,
                        scale=float(ACT_SCALE), bias=bias_ap[:, :],
                    )
                    return et, None
                # poly path: the PSUM-freeing drain is emitted NOW (so the
                # ps_sc buffer recycles fast); the SBUF-only tail is deferred.
                # (the drain must be a copy: TensorTensor cannot read PSUM
                # twice, so the square runs in bf16 afterwards at 2x)
                u = poly.tile([128, GC * QB], bf16, tag="pu", name="pu_t")
                nc.vector.tensor_copy(u[:, :], ps[:, :])

                def tail():
                    z = poly.tile([128, GC * QB], bf16, tag="pz", name="pz_t")
                    nc.vector.tensor_mul(z[:, :], u[:, :], u[:, :])
                    w = poly.tile([128, GC * QB], bf16, tag="pw", name="pw_t")
                    nc.vector.tensor_scalar(w[:, :], z[:, :], float(A3), None, Alu.add)
                    nc.vector.tensor_mul(et[:, :], w[:, :], w[:, :])
                return et, tail

            def attnv_group(h, acc, ets, g, first, last):
                for i in range(GC):
                    m = g * GC + i
                    nc.tensor.matmul(
                        acc[0:VP, :],
                        vext[:, m, h * VP:h * VP + VP],
                        ets[g][:, i * QB:(i + 1) * QB],
                        start=(first and i == 0), stop=(last and i == GC - 1),
                        skip_group_check=True,
                    )

            out_r = out_d.rearrange("(b p) o -> p b o", p=128)

            def epilogue(h, c, acc):
                # numerator+denominator to bf16 via ScalarE (copy shares the
                # exp act table, and Act has slack), transpose on the idle SP
                # DMA xbar, then normalize per 128-query tile
                so = tmp.tile([VP, QB], bf16, tag="so", name="so_t")
                nc.scalar.activation(
                    so[:, :], acc[0:VP, :], mybir.ActivationFunctionType.Copy,
                )
                soT = tmp.tile([128, 4, VP], bf16, tag="soT", name="soT_t")
                nc.sync.dma_start_transpose(soT[:, :, :], so[:, :])
                rec = tmp.tile([128, 4], fp32, tag="rec", name="rec_t")
                nc.vector.reciprocal(rec[:, :], soT[:, :, DH])
                for t in range(4):
                    nc.vector.tensor_scalar_mul(
                        out_sb[:, c * 4 + t, h * DH:(h + 1) * DH],
                        soT[:, t, 0:DH], rec[:, t:t + 1],
                    )
                # stream this (h, c) slab out now instead of one tail DMA
                nc.sync.dma_start(
                    out=out_r[:, c * 4:(c + 1) * 4, h * DH:(h + 1) * DH],
                    in_=out_sb[:, c * 4:(c + 1) * 4, h * DH:(h + 1) * DH],
                )

            # ---- software-pipelined main loop: attnv for iteration i-1 is
            # interleaved group-by-group with scores/exp of iteration i so PE
            # fills its exp-wait gaps and the exp engines never starve.
            prev = None            # (h, c, ets) awaiting attnv
            next_kproj = 0         # k-projection chunks emitted so far
            LAG = 3                # final iteration: attnv trails its own
                                   # scores by LAG groups (exp stays ahead)
            for h in range(2):
                for c in range(NQ):
                    last = (h == 1 and c == NQ - 1)
                    if h == 0:
                        if c == 0:
                            k_proj(next_kproj)
                            next_kproj += 1
                        q_proj(c)
                    acc = None
                    if prev is not None:
                        acc = ps_sm.tile([128, QB], fp32, tag="small", name="ps_small")
                    acc2 = None
                    if last:
                        acc2 = ps_sm.tile([128, QB], fp32, tag="small", name="ps_small")
                    ets = []
                    pending = None
                    for g in range(NG):
                        while next_kproj < 12 and next_kproj * 4 <= GC * g + GC - 1:
                            k_proj(next_kproj)
                            next_kproj += 1
                        # iterations (0,0)/(0,1) carry the k/v projection
                        # drains on DVE, so they run fewer DVE poly groups
                        if h == 0 and c == 0:
                            poly_set = (5, 13, 21)
                        elif h == 0 and c == 1:
                            poly_set = (3, 9, 15, 21)
                        else:
                            poly_set = POLY_GROUPS
                        et, tail = scores_group(h, c, g, poly_set)
                        ets.append(et)
                        if prev is not None:
                            if h == 0 and c == 1:
                                v_proj(GC * g + 1)
                            attnv_group(prev[0], acc, prev[2], g, g == 0, g == NG - 1)
                        elif h == 0 and c == 0:
                            v_proj(GC * g)
                        if last and g >= LAG:
                            attnv_group(h, acc2, ets, g - LAG, g == LAG, False)
                        if pending is not None:
                            pending()
                        pending = tail
                    if pending is not None:
                        pending()
                    if prev is not None:
                        epilogue(prev[0], prev[1], acc)
                    if last:
                        for g in range(NG - LAG, NG):
                            attnv_group(h, acc2, ets, g, False, g == NG - 1)
                        epilogue(h, c, acc2)
                    prev = (h, c, ets)
    return nc


def _shard_inputs(x, source, Wq, Wk, Wv):
    """Build the 8 per-core input maps (host-side layout prep only)."""
    import ml_dtypes

    bf = ml_dtypes.bfloat16
    x = np.asarray(x, np.float32)
    source = np.asarray(source, np.float32)
    alpha = np.sqrt(PP * SCALE)
    WqT = (np.asarray(Wq, np.float32).T * alpha).copy()   # [in, out], scaled
    WkT = (np.asarray(Wk, np.float32).T * alpha).copy()
    WvT = np.asarray(Wv, np.float32).T.copy()

    def pack_qk(WT, h1, h2):
        # [256, 96]: cols 0:32 = head h1, 32:64 = 0, 64:96 = head h2
        pk = np.zeros((256, 96), np.float32)
        pk[:, 0:32] = WT[:, h1 * DH:(h1 + 1) * DH]
        pk[:, 64:96] = WT[:, h2 * DH:(h2 + 1) * DH]
        return np.ascontiguousarray(
            pk.reshape(2, 128, 96).transpose(1, 0, 2).reshape(128, 192)
        ).astype(bf)

    def pack_v(WT, h1, h2):
        pair = np.concatenate(
            [WT[:, h * DH:(h + 1) * DH] for h in (h1, h2)], axis=1
        )  # [256, 64]
        return np.ascontiguousarray(
            pair.reshape(2, 128, 64).transpose(1, 0, 2).reshape(128, 128)
        ).astype(bf)

    in_maps = []
    for c in range(N_CORES):
        b, hp = c // 4, c % 4
        h1, h2 = 2 * hp, 2 * hp + 1
        in_maps.append({
            "xT": np.ascontiguousarray(x[b].T).astype(bf),
            "srcT": np.ascontiguousarray(source[b].T).astype(bf),
            "wq": pack_qk(WqT, h1, h2),
            "wk": pack_qk(WkT, h1, h2),
            "wv": pack_v(WvT, h1, h2),
        })
    return in_maps


def _gather(results):
    out = np.empty((B, L1, D), np.float32)
    for c in range(N_CORES):
        b, hp = c // 4, c % 4
        out[b, :, hp * 64:(hp + 1) * 64] = results[c]["out"]
    return out


def kernel(x, source, Wq, Wk, Wv):
    import sys
    if "/opt/trn_rl_repo" not in sys.path:
        sys.path.insert(0, "/opt/trn_rl_repo")
    from concourse import bass_utils

    if "nc" not in _CACHE:
        _CACHE["nc"] = _build()
    in_maps = _shard_inputs(x, source, Wq, Wk, Wv)
    res = bass_utils.run_bass_kernel_spmd(
        _CACHE["nc"], in_maps, core_ids=list(range(N_CORES))
    )
    return _gather(res.results)
